# revision 11
# baseline (speedup 1.0000x reference)
"""Causal self-attention (B=8, T=1024, C=768, H=12) on 8 trn2 cores.

Sharding: data parallel over batch — core b handles sequence b end-to-end
(no collectives). Per-core bass/Tile program:

  x [1024,768] --PE transpose--> xT [768,1024]
  qkT[f,t] = (W_qkv.T @ x.T) for q,k features (feature-major, heads stacked
             2-per-partition-tile) ; v[t,f] token-major (+ ones column per
             head so the PV matmul also produces the softmax denominator)
  per head pair (A,B packed in one 128-partition tile):
    S.T[tk,tq] = k @ q.T   (row-group packed, K=64 each)
    E = exp(S.T/8)         (no max subtraction: |scores/8| << 1)
    causal mask on diagonal blocks via affine_select (in place, fill=0)
    U.T[d,tq] (+denom row) = [v|1].T-style matmul accumulated over tk tiles
    normalize: U.T *= 1/denom (reciprocal + partition_broadcast)
  y = U_norm.T.T @ W_o + b_o_eff  (K=1 ones-row matmul adds the bias)

Biases: b_q/b_k applied per-partition during qkT copyback; b_v is absorbed
host-side into b_o_eff = b_o + b_v @ W_o (valid because attn rows sum to 1).
"""

import sys

for _p in ("/opt/trn_rl_repo",):
    if _p not in sys.path:
        sys.path.insert(0, _p)

import numpy as np

T, C, H = 1024, 768, 12
D = C // H  # 64
P = 128
NT = T // P  # 8 token tiles
NCD = C // P  # 6 feature tiles
NPAIR = H // 2  # 6 head pairs
VSEG = 193  # per-pair cols: [vA(64)|1] + [0*32|1|0*31|vB(64)] (den_B row 32)
N_CORES = 8

_F32R = None  # set lazily (mybir import)


def _r(ap):
    """Identity: operands are stored as float32r already."""
    return ap


def build_kernel_body():
    """Returns f(tc, y_ap, ins_dict) emitting the per-core program."""
    import concourse.bass as bass
    from concourse import mybir
    from concourse.masks import make_identity

    global _F32R
    _F32R = mybir.dt.float32r
    f32 = mybir.dt.float32
    f32r = mybir.dt.float32r
    AF = mybir.ActivationFunctionType

    def body(tc, y_ap, ins, dbg=None):
        nc = tc.nc
        x_ap = ins["x"]
        wqkv_ap = ins["w_qkv"]
        bqk_ap = ins["b_qk"]
        wo_ap = ins["w_o"]
        bo_ap = ins["b_o_eff"]

        from contextlib import ExitStack

        with ExitStack() as ctx:
            consts = ctx.enter_context(tc.tile_pool(name="consts", bufs=1))
            ident = consts.tile([P, P], f32, tag="ident")
            make_identity(nc, ident)
            ones_f = consts.tile([1, P], f32, tag="ones_f")
            nc.vector.memset(ones_f, 1.0)
            ones_row = consts.tile([1, P], f32r, tag="ones_row")
            nc.scalar.activation(ones_row, ones_f, AF.Identity)
            # constant cols for v_aug: per pair, cols 64..128 = [1 | 0*32 | 1 | 0*31]
            cpat = consts.tile([P, NPAIR, 65], f32, tag="cpat")
            nc.vector.memset(cpat, 0.0)
            nc.vector.memset(cpat[:, :, 0:1], 1.0)
            nc.vector.memset(cpat[:, :, 33:34], 1.0)
            bqk_sb = consts.tile([P, 2 * NCD], f32, tag="bqk")
            nc.sync.dma_start(out=bqk_sb, in_=bqk_ap)

            persist = ctx.enter_context(tc.tile_pool(name="persist", bufs=1))
            xT = [persist.tile([P, T], f32r, tag=f"xT{j}", name=f"xT{j}") for j in range(NCD)]
            qkT = [persist.tile([P, T], f32r, tag=f"qkT{f}", name=f"qkT{f}") for f in range(2 * NCD)]
            v_aug = [
                persist.tile([P, NPAIR * VSEG], f32r, tag=f"vaug{t}", name=f"vaug{t}") for t in range(NT)
            ]
            uT = [persist.tile([P, T], f32r, tag=f"uT{p}", name=f"uT{p}") for p in range(NPAIR)]

            # ---- Phase 1: transpose x into xT (PE transpose, 4 per bank) ----
            with (
                tc.tile_pool(name="xin", bufs=3) as xin,
                tc.tile_pool(name="tp_ps", bufs=3, space="PSUM") as tp_ps,
            ):
                for tg in range(2):  # groups of 4 token tiles
                    xg = xin.tile([P, 4, C], f32, tag="xg")
                    nc.sync.dma_start(
                        out=xg,
                        in_=x_ap.rearrange("(g p) c -> p g c", p=P)[
                            :, 4 * tg : 4 * tg + 4, :
                        ],
                    )
                    for j in range(NCD):
                        ps = tp_ps.tile([P, 4 * P], f32, tag="tp")
                        for s in range(4):
                            nc.tensor.transpose(
                                ps[:, s * P : (s + 1) * P],
                                xg[:, s, j * P : (j + 1) * P],
                                ident,
                            )
                        nc.scalar.activation(
                            xT[j][:, 512 * tg : 512 * (tg + 1)], ps, AF.Identity
                        )

            # ---- Phase 2: qkT (feature-major) and v (token-major) ----
            with (
                tc.tile_pool(name="wq", bufs=1) as wq,
                tc.tile_pool(name="mm_ps", bufs=4, space="PSUM") as mm_ps,
            ):
                w_sb = [wq.tile([P, 3 * C], f32r, tag=f"w{j}", name=f"w{j}") for j in range(NCD)]
                for j in range(NCD):
                    nc.sync.dma_start(
                        out=w_sb[j], in_=wqkv_ap[j * P : (j + 1) * P, :].bitcast(f32r)
                    )

                # qkT[f][:, chunk] = sum_j W[j][:, f*128:...].T @ xT[j][:, chunk]
                for f in range(2 * NCD):
                    for ch in range(2):
                        ps = mm_ps.tile([P, 512], f32, tag="qk_ps")
                        for j in range(NCD):
                            nc.tensor.matmul(
                                ps,
                                _r(w_sb[j][:, f * P : (f + 1) * P]),
                                _r(xT[j][:, 512 * ch : 512 * (ch + 1)]),
                                start=(j == 0),
                                stop=(j == NCD - 1),
                            )
                        nc.scalar.activation(
                            qkT[f][:, 512 * ch : 512 * (ch + 1)],
                            ps,
                            AF.Identity,
                            bias=bqk_sb[:, f : f + 1],
                        )

                # v[t][:, 384-chunk] = sum_j xT[j][:,t*128:..].T @ Wv[j][:, chunk]
                for t in range(NT):
                    va = v_aug[t].rearrange("p (h s) -> p h s", s=VSEG)
                    # constant cols (ones + zero padding), rounded to f32r
                    nc.scalar.activation(va[:, :, 64:129], cpat, AF.Identity)
                    for ch in range(2):
                        ps = mm_ps.tile([P, 384], f32, tag="v_ps")
                        for j in range(NCD):
                            nc.tensor.matmul(
                                ps,
                                _r(xT[j][:, t * P : (t + 1) * P]),
                                _r(w_sb[j][:, 2 * C + 384 * ch : 2 * C + 384 * (ch + 1)]),
                                start=(j == 0),
                                stop=(j == NCD - 1),
                            )
                        psv = ps.rearrange("p (k d) -> p k d", d=D)  # 3 pairs x (A,B)
                        pr = psv.rearrange("p (q a) d -> p q a d", a=2)
                        base = 3 * ch
                        # even heads -> A slot, odd heads -> B slot
                        nc.vector.tensor_copy(
                            va[:, base : base + 3, 0:64], pr[:, :, 0, :]
                        )
                        nc.vector.tensor_copy(
                            va[:, base : base + 3, 129:193], pr[:, :, 1, :]
                        )

            # ---- Phase 3: attention per head pair ----
            with (
                tc.tile_pool(name="s_ps", bufs=2, space="PSUM") as s_ps,
                tc.tile_pool(name="u_ps", bufs=2, space="PSUM") as u_ps,
                tc.tile_pool(name="esb", bufs=3) as esb,
                tc.tile_pool(name="rsb", bufs=2) as rsb,
            ):
                for p in range(NPAIR):
                    kt = qkT[NCD + p]
                    qt = qkT[p]
                    vofs = p * VSEG
                    for j in range(2):  # tq chunks of 512
                        ups_a = u_ps.tile([P, 512], f32, tag="ups_a")
                        ups_b = u_ps.tile([P, 512], f32, tag="ups_b")
                        n_i = 4 * j + 4
                        for i in range(n_i):
                            rel = i * P - 512 * j
                            s0 = 256 if rel >= 256 else 0
                            w = 512 - s0
                            c0 = 512 * j + s0  # global tq col of window start
                            sa = s_ps.tile([P, 512], f32, tag="sa")
                            sb_ = s_ps.tile([P, 512], f32, tag="sb")
                            nc.tensor.matmul(
                                sa[:, :w],
                                _r(kt[0:64, i * P : (i + 1) * P]),
                                _r(qt[0:64, c0 : c0 + w]),
                                start=True,
                                stop=True,
                            )
                            nc.tensor.matmul(
                                sb_[:, :w],
                                _r(kt[64:128, i * P : (i + 1) * P]),
                                _r(qt[64:128, c0 : c0 + w]),
                                start=True,
                                stop=True,
                                tile_position=(64, 0),
                            )
                            ea = esb.tile([P, 512], f32r, tag="ea")
                            eb = esb.tile([P, 512], f32r, tag="eb")
                            nc.scalar.activation(
                                ea[:, :w], sa[:, :w], AF.Exp, scale=0.125
                            )
                            nc.scalar.activation(
                                eb[:, :w], sb_[:, :w], AF.Exp, scale=0.125
                            )
                            if dbg is not None and p == 0 and j == 0 and i == 0:
                                nc.sync.dma_start(out=dbg["e_a"].bitcast(f32r), in_=ea)
                            dend = (i + 1) * P - c0
                            if dend > 0:  # causal mask on the diagonal window
                                for e in (ea, eb):
                                    nc.gpsimd.affine_select(
                                        out=e[:, :dend],
                                        in_=e[:, :dend],
                                        compare_op=mybir.AluOpType.is_ge,
                                        fill=0.0,
                                        base=c0 - i * P,
                                        pattern=[[1, dend]],
                                        channel_multiplier=-1,
                                    )
                            nc.tensor.matmul(
                                ups_a[0:65, s0 : s0 + w],
                                _r(v_aug[i][:, vofs : vofs + 65]),
                                _r(ea[:, :w]),
                                start=(i == 0),
                                stop=(i == n_i - 1),
                            )
                            nc.tensor.matmul(
                                ups_b[:, s0 : s0 + w],
                                _r(v_aug[i][:, vofs + 65 : vofs + 193]),
                                _r(eb[:, :w]),
                                start=(i == 0),
                                stop=(i == n_i - 1),
                            )
                        # normalize: uT[p][:, chunk] = U / denom
                        rec = rsb.tile([P, 512], f32, tag="rec")
                        nc.vector.reciprocal(rec[64:65, :], ups_a[64:65, :])
                        nc.vector.reciprocal(rec[32:33, :], ups_b[32:33, :])
                        rr = rsb.tile([P, 512], f32, tag="rr")

                        def _rep(row_ap):
                            return bass.AP(
                                tensor=row_ap.tensor,
                                offset=row_ap.offset,
                                ap=[list(row_ap.ap[0]), [0, 64], [1, 512]],
                            )

                        nc.gpsimd.dma_start(
                            out=rr[0:64, :], in_=_rep(rec[64:65, :])
                        )
                        nc.gpsimd.dma_start(
                            out=rr[64:128, :], in_=_rep(rec[32:33, :])
                        )
                        nc.vector.tensor_mul(
                            uT[p][0:64, 512 * j : 512 * (j + 1)],
                            ups_a[0:64, :],
                            rr[0:64, :],
                        )
                        nc.vector.tensor_mul(
                            uT[p][64:128, 512 * j : 512 * (j + 1)],
                            ups_b[64:128, :],
                            rr[64:128, :],
                        )
                        if dbg is not None and p == 0 and j == 0:
                            nc.sync.dma_start(out=dbg["rec0"], in_=rec)
                            nc.sync.dma_start(out=dbg["rr0"], in_=rr)
                            ua = rsb.tile([P, 512], f32, tag="ua_dump")
                            nc.vector.tensor_copy(ua[0:65, :], ups_a[0:65, :])
                            ub = rsb.tile([P, 512], f32, tag="ub_dump")
                            nc.vector.tensor_copy(ub[0:128, :], ups_b[0:128, :])
                            nc.sync.dma_start(out=dbg["ups_a0"], in_=ua)
                            nc.sync.dma_start(out=dbg["ups_b0"], in_=ub)

            # ---- Phase 4: output projection + bias ----
            with (
                tc.tile_pool(name="wo", bufs=1) as wo,
                tc.tile_pool(name="p_ps", bufs=4, space="PSUM") as p_ps,
                tc.tile_pool(name="ysb", bufs=3) as ysb,
            ):
                wo_sb = [wo.tile([P, C], f32r, tag=f"wo{j}", name=f"wo{j}") for j in range(NCD)]
                for j in range(NCD):
                    nc.sync.dma_start(out=wo_sb[j], in_=wo_ap[j * P : (j + 1) * P, :].bitcast(f32r))
                bo_sb = wo.tile([1, C], f32r, tag="bo")
                nc.sync.dma_start(out=bo_sb, in_=bo_ap.bitcast(f32r))

                for t in range(NT):
                    yt = ysb.tile([P, C], f32, tag="yt")
                    for ch in range(2):
                        ps = p_ps.tile([P, 384], f32, tag="y_ps")
                        for j in range(NCD):
                            nc.tensor.matmul(
                                ps,
                                _r(uT[j][:, t * P : (t + 1) * P]),
                                _r(wo_sb[j][:, 384 * ch : 384 * (ch + 1)]),
                                start=(j == 0),
                                stop=False,
                            )
                        nc.tensor.matmul(
                            ps,
                            _r(ones_row),
                            _r(bo_sb[:, 384 * ch : 384 * (ch + 1)]),
                            start=False,
                            stop=True,
                        )
                        nc.scalar.activation(
                            yt[:, 384 * ch : 384 * (ch + 1)], ps, AF.Identity
                        )
                    nc.sync.dma_start(
                        out=y_ap[t * P : (t + 1) * P, :], in_=yt
                    )
            if dbg is not None:
                nc.sync.dma_start(out=dbg["xT0"].bitcast(f32r), in_=xT[0])
                nc.sync.dma_start(out=dbg["qk0"].bitcast(f32r), in_=qkT[0])
                nc.sync.dma_start(out=dbg["qk6"].bitcast(f32r), in_=qkT[NCD])
                nc.sync.dma_start(out=dbg["va0"].bitcast(f32r), in_=v_aug[0])
                nc.sync.dma_start(out=dbg["ut0"].bitcast(f32r), in_=uT[0])

    return body


def _host_inputs(x, W_qkv, b_qkv, W_o, b_o):
    """Split/derive per-core host arrays."""
    x = np.ascontiguousarray(np.asarray(x, dtype=np.float32))
    W_qkv = np.ascontiguousarray(np.asarray(W_qkv, dtype=np.float32))
    b_qkv = np.asarray(b_qkv, dtype=np.float32)
    W_o = np.ascontiguousarray(np.asarray(W_o, dtype=np.float32))
    b_o = np.asarray(b_o, dtype=np.float32)
    b_qk = np.ascontiguousarray(b_qkv[: 2 * C].reshape(2 * NCD, P).T)  # [128, 12]
    b_o_eff = (b_o + b_qkv[2 * C :] @ W_o).reshape(1, C).astype(np.float32)
    return x, W_qkv, b_qk, W_o, b_o_eff


_CACHED = {}

DBG_SHAPES = {
    "e_a": (P, 512),
    "rec0": (P, 512),
    "rr0": (P, 512),
    "ups_a0": (P, 512),
    "ups_b0": (P, 512),
    "xT0": (P, T),
    "qk0": (P, T),
    "qk6": (P, T),
    "va0": (P, NPAIR * VSEG),
    "ut0": (P, T),
}


def _build_nc(debug=False):
    key = "nc_dbg" if debug else "nc"
    if key in _CACHED:
        return _CACHED[key]
    from concourse import bacc, mybir
    import concourse.tile as tile

    f32 = mybir.dt.float32
    nc = bacc.Bacc(
        "TRN2",
        target_bir_lowering=False,
        debug=False,
        enable_asserts=False,
        num_devices=N_CORES,
    )
    x_d = nc.dram_tensor("x", [T, C], f32, kind="ExternalInput")
    wqkv_d = nc.dram_tensor("w_qkv", [C, 3 * C], f32, kind="ExternalInput")
    bqk_d = nc.dram_tensor("b_qk", [P, 2 * NCD], f32, kind="ExternalInput")
    wo_d = nc.dram_tensor("w_o", [C, C], f32, kind="ExternalInput")
    bo_d = nc.dram_tensor("b_o_eff", [1, C], f32, kind="ExternalInput")
    y_d = nc.dram_tensor("y", [T, C], f32, kind="ExternalOutput")
    dbg = None
    if debug:
        dbg = {
            k: nc.dram_tensor(f"dbg_{k}", list(s), f32, kind="ExternalOutput").ap()
            for k, s in DBG_SHAPES.items()
        }

    body = build_kernel_body()
    with tile.TileContext(nc) as tc:
        body(
            tc,
            y_d.ap(),
            {
                "x": x_d.ap(),
                "w_qkv": wqkv_d.ap(),
                "b_qk": bqk_d.ap(),
                "w_o": wo_d.ap(),
                "b_o_eff": bo_d.ap(),
            },
            dbg=dbg,
        )
    nc.compile()
    _CACHED[key] = nc
    return nc


def kernel(x, W_qkv, b_qkv, W_o, b_o, _trace=False, _debug=False):
    from concourse.bass_utils import run_bass_kernel_spmd

    x, W_qkv, b_qk, W_o, b_o_eff = _host_inputs(x, W_qkv, b_qkv, W_o, b_o)
    nc = _build_nc(debug=_debug)
    in_maps = [
        {
            "x": x[b],
            "w_qkv": W_qkv,
            "b_qk": b_qk,
            "w_o": W_o,
            "b_o_eff": b_o_eff,
        }
        for b in range(N_CORES)
    ]
    res = run_bass_kernel_spmd(
        nc, in_maps, core_ids=list(range(N_CORES)), trace=_trace
    )
    _CACHED["last_results"] = res
    out = np.stack([res.results[b]["y"] for b in range(N_CORES)], axis=0)
    return out.astype(np.float32)
